# revision 18
# baseline (speedup 1.0000x reference)
"""Trainium2 Bass kernel for fused MHA block (nn_MultiHeadAttention_7636451852747).

Reference math (B=2, S=4096, D=512, H=8, hd=64):
    q = (x @ Wq + bq).reshape(B, H, S, hd)   # torch-style .view, no transpose!
    ... scores = q @ k^T / 8; attn = softmax(scores) @ v -> reshape(B,S,D)
    y = LayerNorm(x + attn) * gamma + beta

Key structural insight: the .view(B,H,S,hd) reshape (without transpose) means
head h of batch b only reads rows [h*512, (h+1)*512) of x[b].  The problem
therefore decomposes into B*H = 16 fully independent [512,512] chunks; each of
the 8 cores processes 2 chunks end-to-end with zero inter-core communication.

Within a chunk (x_c = x[b, h*512:(h+1)*512, :], shape [512, 512]):
    q = x_c Wq + bq viewed as Q[4096, 64] with Q[8s+j, d] = q[s, 64j+d]
    scores^T tiles: S_T[(jk,r)][p, s_q] for nk = 8*(128r+p)+jk, nq = 8*s_q+jq
      = matmul(lhsT=k^T[64jk:+64, 128r:+128], rhs=q^T[64jq:+64, :])
    E = exp(S_T/8) in bf16; attn^T strip = sum over (jk,r) of
      matmul(lhsT=[V_tile | ones], rhs=E) -> [65, 512] psum; row 64 = softmax
      denominator (ones-column trick).  PE-transpose [65,128] blocks back to
      natural layout, divide by denominator, add residual, LayerNorm on DVE
      (Newton rsqrt to keep ACT free for exp, which is the bottleneck engine).
All matmuls are bf16 with fp32 PSUM accumulation.
"""
import os
import numpy as np
import ml_dtypes
from contextlib import ExitStack

BF16 = None  # set in _imports
_STATE = {}


def _imports():
    global bass, bacc, tile, mybir, bass_utils, F32, BF16, I32, ALU, ACTF
    import concourse.bass as bass
    import concourse.bacc as bacc
    import concourse.tile as tile
    from concourse import mybir
    from concourse import bass_utils
    F32 = mybir.dt.float32
    BF16 = mybir.dt.bfloat16
    I32 = mybir.dt.int32
    ALU = mybir.AluOpType
    ACTF = mybir.ActivationFunctionType


N_CORES = 8
CHUNKS_PER_CORE = 2
S = 512          # rows per chunk
D = 512          # model dim
HD = 64          # head dim of the viewed [4096, 64] matrices
NQ = 4096        # sub-rows per chunk (S*D/HD)
EPS = 1e-5


def _emit(nc, tc, ctx):
    F32l, BF16l, I32l = F32, BF16, I32
    x_d = nc.dram_tensor("xc", [CHUNKS_PER_CORE, S, D], F32l, kind="ExternalInput").ap()
    xb_d = nc.dram_tensor("xcb", [CHUNKS_PER_CORE, S, D], BF16l, kind="ExternalInput").ap()
    w_d = {n: nc.dram_tensor(n, [D, D], BF16l, kind="ExternalInput").ap()
           for n in ("wq", "wk", "wv")}
    b_d = {n: nc.dram_tensor(n, [1, D], BF16l, kind="ExternalInput").ap()
           for n in ("bq", "bk", "bv")}
    ones_d = nc.dram_tensor("ones", [1, D], BF16l, kind="ExternalInput").ap()
    idf_d = nc.dram_tensor("idf", [128, 128], F32l, kind="ExternalInput").ap()
    gb_d = nc.dram_tensor("gb", [128, D], F32l, kind="ExternalInput").ap()
    bb_d = nc.dram_tensor("bb", [128, D], F32l, kind="ExternalInput").ap()
    y_d = nc.dram_tensor("y", [CHUNKS_PER_CORE, S, D], F32l, kind="ExternalOutput").ap()

    # pools
    consts = ctx.enter_context(tc.tile_pool(name="consts", bufs=1))
    chunkp = ctx.enter_context(tc.tile_pool(name="chunk", bufs=2))
    epool = ctx.enter_context(tc.tile_pool(name="epool", bufs=8))
    attp = ctx.enter_context(tc.tile_pool(name="attp", bufs=2))
    ypool = ctx.enter_context(tc.tile_pool(name="ypool", bufs=3))
    small = ctx.enter_context(tc.tile_pool(name="small", bufs=4))
    # PSUM budget (8 banks): score 2x[128,1024]=4, attn 2, proj 2 (shared
    # with the finalize transposes via the same tag)
    ps_proj = ctx.enter_context(tc.tile_pool(name="ps_proj", bufs=2, space="PSUM"))
    ps_score = ctx.enter_context(tc.tile_pool(name="ps_score", bufs=2, space="PSUM"))
    ps_attn = ctx.enter_context(tc.tile_pool(name="ps_attn", bufs=2, space="PSUM"))

    # ---- constants to SBUF
    w_sb = {}
    for n in ("wq", "wk", "wv"):  # wq/wk first: they gate the first projections
        t = consts.tile([128, 4 * D], BF16l, tag=n, name=f"w_{n}")
        for mt in range(4):
            nc.sync.dma_start(t[:, 512 * mt:512 * (mt + 1)], w_d[n][128 * mt:128 * (mt + 1), :])
        w_sb[n] = t
    b_sb = {}
    for n in ("bq", "bk", "bv"):
        t = consts.tile([1, D], BF16l, tag=n, name=f"b_{n}")
        nc.sync.dma_start(t[:], b_d[n][:])
        b_sb[n] = t
    ones = consts.tile([1, D], BF16l, tag="ones")
    nc.sync.dma_start(ones[:], ones_d[:])
    idf = consts.tile([128, 128], F32l, tag="idf")
    nc.sync.dma_start(idf[:], idf_d[:])
    gb = consts.tile([128, D], F32l, tag="gb")
    nc.sync.dma_start(gb[:], gb_d[:])
    bb = consts.tile([128, D], F32l, tag="bb")
    nc.sync.dma_start(bb[:], bb_d[:])

    st = [{} for _ in range(CHUNKS_PER_CORE)]  # per-chunk tile state

    def prep_load(c):
        """DMA x; x^T in one hardware DMA transpose (XBAR, bf16).
        dma_start_transpose into a [p, mt, s] view lands source row m at
        partition m%128 of slab m//128 -- exactly the m-tile-major layout."""
        s = st[c]
        s["xf"] = xf = chunkp.tile([128, 4 * D], F32l, tag="xf", name=f"xf{c}")
        for t in range(4):
            nc.sync.dma_start(xf[:, 512 * t:512 * (t + 1)], x_d[c, 128 * t:128 * (t + 1), :])
        s["xT"] = xT = chunkp.tile([128, 4 * D], BF16l, tag="xT", name=f"xT{c}")
        for mt in range(4):
            nc.sync.dma_start_transpose(
                xT[:, 512 * mt:512 * (mt + 1)], xb_d[c][:, 128 * mt:128 * (mt + 1)])
        s["qT"] = chunkp.tile([128, 4 * D], BF16l, tag="qT", name=f"qT{c}")
        s["qTs"] = chunkp.tile([128, 4 * D], BF16l, tag="qTs", name=f"qTs{c}")
        s["kT"] = chunkp.tile([128, 4 * D], BF16l, tag="kT", name=f"kT{c}")
        s["vp"] = chunkp.tile([128, 4 * 520], BF16l, tag="vp", name=f"vp{c}")
        s["h"] = chunkp.tile([128, 4 * D], F32l, tag="h", name=f"h{c}")

    def prep_qk(c, t, which):
        """One q^T or k^T projection column tile (plus qTs swap for q)."""
        s = st[c]
        xT, qT, qTs, kT = s["xT"], s["qT"], s["qTs"], s["kT"]
        wname, bname, dst = (("wq", "bq", qT) if which == "q" else ("wk", "bk", kT))
        pp = ps_proj.tile([128, D], F32l, tag="proj", name=f"pp{c}_{wname}{t}")
        for mt in range(4):
            nc.tensor.matmul(
                pp[:],
                w_sb[wname][:, 512 * mt + 128 * t:512 * mt + 128 * t + 128],
                xT[:, 512 * mt:512 * (mt + 1)],
                start=(mt == 0), stop=False)
        nc.tensor.matmul(pp[:], b_sb[bname][0:1, 128 * t:128 * (t + 1)],
                         ones[0:1, :], start=False, stop=True)
        nc.vector.tensor_copy(dst[0:64, 512 * t:512 * (t + 1)], pp[0:64, :])
        nc.vector.tensor_copy(dst[64:128, 512 * t:512 * (t + 1)], pp[64:128, :])
        if which == "q":
            nc.sync.dma_start(qTs[64:128, 512 * t:512 * (t + 1)], qT[0:64, 512 * t:512 * (t + 1)])
            nc.sync.dma_start(qTs[0:64, 512 * t:512 * (t + 1)], qT[64:128, 512 * t:512 * (t + 1)])

    def prep_v(c, t):
        s = st[c]
        xT, vp = s["xT"], s["vp"]
        pp = ps_proj.tile([128, D], F32l, tag="proj", name=f"pp{c}_v{t}")
        for mt in range(4):
            nc.tensor.matmul(pp[:], xT[:, 512 * mt + 128 * t:512 * mt + 128 * t + 128],
                             w_sb["wv"][:, 512 * mt:512 * (mt + 1)],
                             start=(mt == 0), stop=False)
        nc.tensor.matmul(pp[:], ones[0:1, 0:128], b_sb["bv"][0:1, :],
                         start=False, stop=True)
        blk = vp[:, 520 * t:520 * (t + 1)].rearrange("p (j c) -> p j c", c=65)
        nc.vector.tensor_copy(blk[:, :, 0:64], pp[:].rearrange("p (j c) -> p j c", c=64))
        nc.vector.memset(blk[:, :, 64], 1.0)

    def prep_qkv(c, t):
        prep_qk(c, t, "q")
        prep_qk(c, t, "k")
        prep_v(c, t)

    def strips(c, jp):
        """One jq-pair: scores (row-packed), 1024-wide exp, attn accumulate,
        transpose back + residual."""
        s = st[c]
        qT, qTs, kT, vp, xf, h = s["qT"], s["qTs"], s["kT"], s["vp"], s["xf"], s["h"]

        def qrhs(jq, par):
            src = qT if (jq % 2) == par else qTs
            return src[64 * par:64 * par + 64, 512 * (jq // 2):512 * (jq // 2) + 512]

        jq0, jq1 = 2 * jp, 2 * jp + 1
        pa = [ps_attn.tile([65, D], F32l, tag="attn", name=f"pa{c}_{jp}_{i}")
              for i in range(2)]
        for r in range(4):
            for jku in range(4):
                jk0, jk1 = 2 * jku, 2 * jku + 1
                koff = 512 * jku + 128 * r
                ps0 = ps_score.tile([128, 2 * D], F32l, tag="sps", name=f"s0_{c}_{jp}_{r}_{jku}")
                ps1 = ps_score.tile([128, 2 * D], F32l, tag="sps", name=f"s1_{c}_{jp}_{r}_{jku}")
                nc.tensor.matmul(ps0[:, 0:512], kT[0:64, koff:koff + 128],
                                 qrhs(jq0, 0), start=True, stop=True,
                                 tile_position=(0, 0))
                nc.tensor.matmul(ps1[:, 0:512], kT[64:128, koff:koff + 128],
                                 qrhs(jq0, 1), start=True, stop=True,
                                 tile_position=(64, 0))
                nc.tensor.matmul(ps0[:, 512:1024], kT[0:64, koff:koff + 128],
                                 qrhs(jq1, 0), start=True, stop=True,
                                 tile_position=(0, 0))
                nc.tensor.matmul(ps1[:, 512:1024], kT[64:128, koff:koff + 128],
                                 qrhs(jq1, 1), start=True, stop=True,
                                 tile_position=(64, 0))
                et0 = epool.tile([128, 2 * D], BF16l, tag="e", name=f"e0_{c}_{jp}_{r}_{jku}")
                et1 = epool.tile([128, 2 * D], BF16l, tag="e", name=f"e1_{c}_{jp}_{r}_{jku}")
                nc.scalar.activation(et0[:], ps0[:], ACTF.Exp, scale=0.125)
                nc.scalar.activation(et1[:], ps1[:], ACTF.Exp, scale=0.125)
                first = (r == 0 and jku == 0)
                last = (r == 3 and jku == 3)
                v0 = vp[:, 520 * r + 65 * jk0:520 * r + 65 * jk0 + 65]
                v1 = vp[:, 520 * r + 65 * jk1:520 * r + 65 * jk1 + 65]
                nc.tensor.matmul(pa[0][:], v0, et0[:, 0:512], start=first,
                                 stop=False, skip_group_check=True)
                nc.tensor.matmul(pa[0][:], v1, et1[:, 0:512], start=False,
                                 stop=last, skip_group_check=True)
                nc.tensor.matmul(pa[1][:], v0, et0[:, 512:1024], start=first,
                                 stop=False, skip_group_check=True)
                nc.tensor.matmul(pa[1][:], v1, et1[:, 512:1024], start=False,
                                 stop=last, skip_group_check=True)
        for half, jq in ((0, jq0), (1, jq1)):
            asb = attp.tile([65, D], F32l, tag="asb", name=f"asb{c}_{jp}_{half}")
            nc.vector.tensor_copy(asb[:], pa[half][:])
            for b in range(4):
                tps = ps_proj.tile([128, 65], F32l, tag="proj", name=f"atr{c}_{jp}_{half}_{b}")
                nc.tensor.transpose(tps[:], asb[0:65, 128 * b:128 * (b + 1)],
                                    idf[0:65, 0:65])
                rcp = small.tile([128, 1], F32l, tag="rcp", name=f"rcp{c}_{jp}_{half}_{b}")
                nc.vector.reciprocal(rcp[:], tps[:, 64:65])
                nc.vector.scalar_tensor_tensor(
                    h[:, 512 * b + 64 * jq:512 * b + 64 * jq + 64],
                    tps[:, 0:64], rcp[:],
                    xf[:, 512 * b + 64 * jq:512 * b + 64 * jq + 64],
                    op0=ALU.mult, op1=ALU.add)

    def layer_norm(c):
        """LayerNorm on DVE only; Newton rsqrt batched across the 4 s-tiles."""
        s = st[c]
        h = s["h"]
        mvall = small.tile([128, 8], F32l, tag="mvall", name=f"mv{c}")
        for b in range(4):
            st6 = small.tile([128, 6], F32l, tag="st6", name=f"st6_{c}_{b}")
            nc.vector.bn_stats(st6[:], h[:, 512 * b:512 * (b + 1)])
            nc.vector.bn_aggr(mvall[:, 2 * b:2 * b + 2], st6[:])
        mean4 = mvall[:].rearrange("p (b two) -> p b two", two=2)[:, :, 0]
        var4 = mvall[:].rearrange("p (b two) -> p b two", two=2)[:, :, 1]
        t4 = small.tile([128, 4], F32l, tag="t4", name=f"t4_{c}")
        nc.vector.tensor_scalar_add(t4[:], var4, EPS)
        yi = small.tile([128, 4], I32l, tag="yi", name=f"yi{c}")
        nc.vector.tensor_scalar(yi[:], t4[:].bitcast(I32l), 1, None,
                                op0=ALU.arith_shift_right)
        nc.vector.tensor_scalar(yi[:], yi[:], 0x5F3759DF, -1,
                                op0=ALU.subtract, op1=ALU.mult)
        rstd = small.tile([128, 4], F32l, tag="rstd", name=f"rstd{c}")
        nc.vector.tensor_copy(rstd[:], yi[:].bitcast(F32l))
        y2 = small.tile([128, 4], F32l, tag="y2", name=f"y2_{c}")
        dd = small.tile([128, 4], F32l, tag="dd", name=f"dd{c}")
        for _ in range(3):
            nc.vector.tensor_tensor(y2[:], rstd[:], rstd[:], op=ALU.mult)
            nc.vector.tensor_tensor(y2[:], y2[:], t4[:], op=ALU.mult)
            nc.vector.tensor_scalar(dd[:], y2[:], -0.5, 1.5,
                                    op0=ALU.mult, op1=ALU.add)
            nc.vector.tensor_tensor(rstd[:], rstd[:], dd[:], op=ALU.mult)
        bco = small.tile([128, 4], F32l, tag="bco", name=f"bco{c}")
        nc.vector.tensor_tensor(bco[:], mean4, rstd[:], op=ALU.mult)
        nc.vector.tensor_scalar_mul(bco[:], bco[:], -1.0)
        for b in range(4):
            yt = ypool.tile([128, D], F32l, tag="yt", name=f"yt{c}_{b}")
            nc.vector.tensor_scalar(yt[:], h[:, 512 * b:512 * (b + 1)],
                                    rstd[:, b:b + 1], bco[:, b:b + 1],
                                    op0=ALU.mult, op1=ALU.add)
            nc.vector.tensor_tensor(yt[:], yt[:], gb[:], op=ALU.mult)
            nc.vector.tensor_tensor(yt[:], yt[:], bb[:], op=ALU.add)
            nc.sync.dma_start(y_d[c, 128 * b:128 * (b + 1), :], yt[:])

    # ---- emission schedule: stagger chunk-1 prep into chunk-0's strips so
    # the PE fills ACT-idle gaps with the next chunk's projections.
    prep_load(0)
    for t in range(4):
        prep_qkv(0, t)
    strips(0, 0)
    prep_load(1)
    strips(0, 1)
    prep_qk(1, 0, "q")
    prep_qk(1, 0, "k")
    prep_v(1, 0)
    prep_qk(1, 1, "q")
    strips(0, 2)
    prep_qk(1, 1, "k")
    prep_v(1, 1)
    prep_qk(1, 2, "q")
    prep_qk(1, 2, "k")
    strips(0, 3)
    prep_v(1, 2)
    prep_qk(1, 3, "q")
    prep_qk(1, 3, "k")
    prep_v(1, 3)
    layer_norm(0)
    for jp in range(4):
        strips(1, jp)
    layer_norm(1)


def build():
    """Build + compile the Bass module (cached)."""
    if "nc" in _STATE:
        return _STATE["nc"]
    _imports()
    nc = bacc.Bacc("TRN2", target_bir_lowering=False, debug=False,
                   num_devices=N_CORES)
    with tile.TileContext(nc) as tc:
        with ExitStack() as ctx:
            _emit(nc, tc, ctx)
    nc.compile()
    _STATE["nc"] = nc
    return nc


def host_inputs(Wq, bq, Wk, bk, Wv, bv, gamma, beta):
    """Shared per-core constant inputs (everything except x chunks)."""
    bf = ml_dtypes.bfloat16
    base = {
        "wq": np.asarray(Wq, np.float32).astype(bf),
        "wk": np.asarray(Wk, np.float32).astype(bf),
        "wv": np.asarray(Wv, np.float32).astype(bf),
        "bq": np.asarray(bq, np.float32).reshape(1, D).astype(bf),
        "bk": np.asarray(bk, np.float32).reshape(1, D).astype(bf),
        "bv": np.asarray(bv, np.float32).reshape(1, D).astype(bf),
        "ones": np.ones((1, D), bf),
        "idf": np.eye(128, dtype=np.float32),
        "gb": np.broadcast_to(np.asarray(gamma, np.float32), (128, D)).copy(),
        "bb": np.broadcast_to(np.asarray(beta, np.float32), (128, D)).copy(),
    }
    return base


def kernel(x, Wq, bq, Wk, bk, Wv, bv, gamma, beta):
    _imports()
    nc = build()
    x = np.asarray(x, np.float32)
    B, Sfull, Dm = x.shape
    chunks = x.reshape(B * 8, S, D)  # chunk c = (b = c//8, head = c%8)
    bf = ml_dtypes.bfloat16
    base = host_inputs(Wq=Wq, bq=bq, Wk=Wk, bk=bk, Wv=Wv, bv=bv,
                       gamma=gamma, beta=beta)
    in_maps = []
    for i in range(N_CORES):
        xc = np.ascontiguousarray(chunks[2 * i:2 * i + 2])
        m = dict(base)
        m["xc"] = xc
        m["xcb"] = xc.astype(bf)
        in_maps.append(m)
    res = bass_utils.run_bass_kernel_spmd(nc, in_maps, core_ids=list(range(N_CORES)))
    out_chunks = np.empty((B * 8, S, D), np.float32)
    for i in range(N_CORES):
        out_chunks[2 * i:2 * i + 2] = res.results[i]["y"]
    return out_chunks.reshape(B, Sfull, Dm)


# revision 19
# speedup vs baseline: 1.0322x; 1.0322x over previous
"""Trainium2 Bass kernel for fused MHA block (nn_MultiHeadAttention_7636451852747).

Reference math (B=2, S=4096, D=512, H=8, hd=64):
    q = (x @ Wq + bq).reshape(B, H, S, hd)   # torch-style .view, no transpose!
    ... scores = q @ k^T / 8; attn = softmax(scores) @ v -> reshape(B,S,D)
    y = LayerNorm(x + attn) * gamma + beta

Key structural insight: the .view(B,H,S,hd) reshape (without transpose) means
head h of batch b only reads rows [h*512, (h+1)*512) of x[b].  The problem
therefore decomposes into B*H = 16 fully independent [512,512] chunks; each of
the 8 cores processes 2 chunks end-to-end with zero inter-core communication.

Within a chunk (x_c = x[b, h*512:(h+1)*512, :], shape [512, 512]):
    q = x_c Wq + bq viewed as Q[4096, 64] with Q[8s+j, d] = q[s, 64j+d]
    scores^T tiles: S_T[(jk,r)][p, s_q] for nk = 8*(128r+p)+jk, nq = 8*s_q+jq
      = matmul(lhsT=k^T[64jk:+64, 128r:+128], rhs=q^T[64jq:+64, :])
    E = exp(S_T/8) in bf16; attn^T strip = sum over (jk,r) of
      matmul(lhsT=[V_tile | ones], rhs=E) -> [65, 512] psum; row 64 = softmax
      denominator (ones-column trick).  PE-transpose [65,128] blocks back to
      natural layout, divide by denominator, add residual, LayerNorm on DVE
      (Newton rsqrt to keep ACT free for exp, which is the bottleneck engine).
All matmuls are bf16 with fp32 PSUM accumulation.
"""
import os
import numpy as np
import ml_dtypes
from contextlib import ExitStack

BF16 = None  # set in _imports
_STATE = {}


def _imports():
    global bass, bacc, tile, mybir, bass_utils, F32, BF16, I32, ALU, ACTF
    import concourse.bass as bass
    import concourse.bacc as bacc
    import concourse.tile as tile
    from concourse import mybir
    from concourse import bass_utils
    F32 = mybir.dt.float32
    BF16 = mybir.dt.bfloat16
    I32 = mybir.dt.int32
    ALU = mybir.AluOpType
    ACTF = mybir.ActivationFunctionType


N_CORES = 8
CHUNKS_PER_CORE = 2
S = 512          # rows per chunk
D = 512          # model dim
HD = 64          # head dim of the viewed [4096, 64] matrices
NQ = 4096        # sub-rows per chunk (S*D/HD)
EPS = 1e-5


def _emit(nc, tc, ctx):
    F32l, BF16l, I32l = F32, BF16, I32
    x_d = nc.dram_tensor("xc", [CHUNKS_PER_CORE, S, D], F32l, kind="ExternalInput").ap()
    xb_d = nc.dram_tensor("xcb", [CHUNKS_PER_CORE, S, D], BF16l, kind="ExternalInput").ap()
    w_d = {n: nc.dram_tensor(n, [D, D], BF16l, kind="ExternalInput").ap()
           for n in ("wq", "wk", "wv")}
    b_d = {n: nc.dram_tensor(n, [1, D], BF16l, kind="ExternalInput").ap()
           for n in ("bq", "bk", "bv")}
    ones_d = nc.dram_tensor("ones", [1, D], BF16l, kind="ExternalInput").ap()
    idf_d = nc.dram_tensor("idf", [128, 128], F32l, kind="ExternalInput").ap()
    gb_d = nc.dram_tensor("gb", [128, D], F32l, kind="ExternalInput").ap()
    bb_d = nc.dram_tensor("bb", [128, D], F32l, kind="ExternalInput").ap()
    y_d = nc.dram_tensor("y", [CHUNKS_PER_CORE, S, D], F32l, kind="ExternalOutput").ap()

    # pools
    consts = ctx.enter_context(tc.tile_pool(name="consts", bufs=1))
    chunkp = ctx.enter_context(tc.tile_pool(name="chunk", bufs=2))
    epool = ctx.enter_context(tc.tile_pool(name="epool", bufs=8))
    attp = ctx.enter_context(tc.tile_pool(name="attp", bufs=2))
    ypool = ctx.enter_context(tc.tile_pool(name="ypool", bufs=3))
    small = ctx.enter_context(tc.tile_pool(name="small", bufs=4))
    # PSUM budget (8 banks): score 2x[128,1024]=4, attn 2, proj 2 (shared
    # with the finalize transposes via the same tag)
    ps_proj = ctx.enter_context(tc.tile_pool(name="ps_proj", bufs=2, space="PSUM"))
    ps_score = ctx.enter_context(tc.tile_pool(name="ps_score", bufs=2, space="PSUM"))
    ps_attn = ctx.enter_context(tc.tile_pool(name="ps_attn", bufs=2, space="PSUM"))

    # ---- constant tiles (DMAs emitted by _consts_early/_late below so the
    # x-transpose DMAs can go FIRST in the single HWDGE queue: the first
    # projection matmul is gated on x^T, not on the weights)
    w_sb = {n: consts.tile([128, 4 * D], BF16l, tag=n, name=f"w_{n}")
            for n in ("wq", "wk", "wv")}
    b_sb = {n: consts.tile([1, D], BF16l, tag=n, name=f"b_{n}")
            for n in ("bq", "bk", "bv")}
    ones = consts.tile([1, D], BF16l, tag="ones")
    idf = consts.tile([128, 128], F32l, tag="idf")
    gb = consts.tile([128, D], F32l, tag="gb")
    bb = consts.tile([128, D], F32l, tag="bb")

    def consts_early():
        for n in ("wq", "wk"):
            for mt in range(4):
                nc.sync.dma_start(w_sb[n][:, 512 * mt:512 * (mt + 1)],
                                  w_d[n][128 * mt:128 * (mt + 1), :])
        for n in ("bq", "bk"):
            nc.sync.dma_start(b_sb[n][:], b_d[n][:])
        nc.sync.dma_start(ones[:], ones_d[:])

    def consts_late():
        for mt in range(4):
            nc.sync.dma_start(w_sb["wv"][:, 512 * mt:512 * (mt + 1)],
                              w_d["wv"][128 * mt:128 * (mt + 1), :])
        nc.sync.dma_start(b_sb["bv"][:], b_d["bv"][:])
        nc.sync.dma_start(idf[:], idf_d[:])
        nc.sync.dma_start(gb[:], gb_d[:])
        nc.sync.dma_start(bb[:], bb_d[:])

    st = [{} for _ in range(CHUNKS_PER_CORE)]  # per-chunk tile state

    def prep_load(c):
        """DMA x; x^T in one hardware DMA transpose (XBAR, bf16).
        dma_start_transpose into a [p, mt, s] view lands source row m at
        partition m%128 of slab m//128 -- exactly the m-tile-major layout."""
        s = st[c]
        s["xf"] = xf = chunkp.tile([128, 4 * D], F32l, tag="xf", name=f"xf{c}")
        for t in range(4):
            nc.sync.dma_start(xf[:, 512 * t:512 * (t + 1)], x_d[c, 128 * t:128 * (t + 1), :])
        s["xT"] = xT = chunkp.tile([128, 4 * D], BF16l, tag="xT", name=f"xT{c}")
        for mt in range(4):
            nc.sync.dma_start_transpose(
                xT[:, 512 * mt:512 * (mt + 1)], xb_d[c][:, 128 * mt:128 * (mt + 1)])
        s["qT"] = chunkp.tile([128, 4 * D], BF16l, tag="qT", name=f"qT{c}")
        s["qTs"] = chunkp.tile([128, 4 * D], BF16l, tag="qTs", name=f"qTs{c}")
        s["kT"] = chunkp.tile([128, 4 * D], BF16l, tag="kT", name=f"kT{c}")
        s["vp"] = chunkp.tile([128, 4 * 520], BF16l, tag="vp", name=f"vp{c}")
        s["h"] = chunkp.tile([128, 4 * D], F32l, tag="h", name=f"h{c}")

    def prep_qk(c, t, which):
        """One q^T or k^T projection column tile (plus qTs swap for q)."""
        s = st[c]
        xT, qT, qTs, kT = s["xT"], s["qT"], s["qTs"], s["kT"]
        wname, bname, dst = (("wq", "bq", qT) if which == "q" else ("wk", "bk", kT))
        pp = ps_proj.tile([128, D], F32l, tag="proj", name=f"pp{c}_{wname}{t}")
        for mt in range(4):
            nc.tensor.matmul(
                pp[:],
                w_sb[wname][:, 512 * mt + 128 * t:512 * mt + 128 * t + 128],
                xT[:, 512 * mt:512 * (mt + 1)],
                start=(mt == 0), stop=False)
        nc.tensor.matmul(pp[:], b_sb[bname][0:1, 128 * t:128 * (t + 1)],
                         ones[0:1, :], start=False, stop=True)
        nc.vector.tensor_copy(dst[0:64, 512 * t:512 * (t + 1)], pp[0:64, :])
        nc.vector.tensor_copy(dst[64:128, 512 * t:512 * (t + 1)], pp[64:128, :])
        if which == "q":
            nc.sync.dma_start(qTs[64:128, 512 * t:512 * (t + 1)], qT[0:64, 512 * t:512 * (t + 1)])
            nc.sync.dma_start(qTs[0:64, 512 * t:512 * (t + 1)], qT[64:128, 512 * t:512 * (t + 1)])

    def prep_v(c, t):
        s = st[c]
        xT, vp = s["xT"], s["vp"]
        pp = ps_proj.tile([128, D], F32l, tag="proj", name=f"pp{c}_v{t}")
        for mt in range(4):
            nc.tensor.matmul(pp[:], xT[:, 512 * mt + 128 * t:512 * mt + 128 * t + 128],
                             w_sb["wv"][:, 512 * mt:512 * (mt + 1)],
                             start=(mt == 0), stop=False)
        nc.tensor.matmul(pp[:], ones[0:1, 0:128], b_sb["bv"][0:1, :],
                         start=False, stop=True)
        blk = vp[:, 520 * t:520 * (t + 1)].rearrange("p (j c) -> p j c", c=65)
        nc.vector.tensor_copy(blk[:, :, 0:64], pp[:].rearrange("p (j c) -> p j c", c=64))
        nc.vector.memset(blk[:, :, 64], 1.0)

    def prep_qkv(c, t):
        prep_qk(c, t, "q")
        prep_qk(c, t, "k")
        prep_v(c, t)

    def strips(c, jp):
        """One jq-pair: scores (row-packed), 1024-wide exp, attn accumulate,
        transpose back + residual."""
        s = st[c]
        qT, qTs, kT, vp, xf, h = s["qT"], s["qTs"], s["kT"], s["vp"], s["xf"], s["h"]

        def qrhs(jq, par):
            src = qT if (jq % 2) == par else qTs
            return src[64 * par:64 * par + 64, 512 * (jq // 2):512 * (jq // 2) + 512]

        jq0, jq1 = 2 * jp, 2 * jp + 1
        pa = [ps_attn.tile([65, D], F32l, tag="attn", name=f"pa{c}_{jp}_{i}")
              for i in range(2)]
        for r in range(4):
            for jku in range(4):
                jk0, jk1 = 2 * jku, 2 * jku + 1
                koff = 512 * jku + 128 * r
                ps0 = ps_score.tile([128, 2 * D], F32l, tag="sps", name=f"s0_{c}_{jp}_{r}_{jku}")
                ps1 = ps_score.tile([128, 2 * D], F32l, tag="sps", name=f"s1_{c}_{jp}_{r}_{jku}")
                nc.tensor.matmul(ps0[:, 0:512], kT[0:64, koff:koff + 128],
                                 qrhs(jq0, 0), start=True, stop=True,
                                 tile_position=(0, 0))
                nc.tensor.matmul(ps1[:, 0:512], kT[64:128, koff:koff + 128],
                                 qrhs(jq0, 1), start=True, stop=True,
                                 tile_position=(64, 0))
                nc.tensor.matmul(ps0[:, 512:1024], kT[0:64, koff:koff + 128],
                                 qrhs(jq1, 0), start=True, stop=True,
                                 tile_position=(0, 0))
                nc.tensor.matmul(ps1[:, 512:1024], kT[64:128, koff:koff + 128],
                                 qrhs(jq1, 1), start=True, stop=True,
                                 tile_position=(64, 0))
                et0 = epool.tile([128, 2 * D], BF16l, tag="e", name=f"e0_{c}_{jp}_{r}_{jku}")
                et1 = epool.tile([128, 2 * D], BF16l, tag="e", name=f"e1_{c}_{jp}_{r}_{jku}")
                nc.scalar.activation(et0[:], ps0[:], ACTF.Exp, scale=0.125)
                nc.scalar.activation(et1[:], ps1[:], ACTF.Exp, scale=0.125)
                first = (r == 0 and jku == 0)
                last = (r == 3 and jku == 3)
                v0 = vp[:, 520 * r + 65 * jk0:520 * r + 65 * jk0 + 65]
                v1 = vp[:, 520 * r + 65 * jk1:520 * r + 65 * jk1 + 65]
                nc.tensor.matmul(pa[0][:], v0, et0[:, 0:512], start=first,
                                 stop=False, skip_group_check=True)
                nc.tensor.matmul(pa[0][:], v1, et1[:, 0:512], start=False,
                                 stop=last, skip_group_check=True)
                nc.tensor.matmul(pa[1][:], v0, et0[:, 512:1024], start=first,
                                 stop=False, skip_group_check=True)
                nc.tensor.matmul(pa[1][:], v1, et1[:, 512:1024], start=False,
                                 stop=last, skip_group_check=True)
        for half, jq in ((0, jq0), (1, jq1)):
            asb = attp.tile([65, D], F32l, tag="asb", name=f"asb{c}_{jp}_{half}")
            nc.vector.tensor_copy(asb[:], pa[half][:])
            for b in range(4):
                tps = ps_proj.tile([128, 65], F32l, tag="proj", name=f"atr{c}_{jp}_{half}_{b}")
                nc.tensor.transpose(tps[:], asb[0:65, 128 * b:128 * (b + 1)],
                                    idf[0:65, 0:65])
                rcp = small.tile([128, 1], F32l, tag="rcp", name=f"rcp{c}_{jp}_{half}_{b}")
                nc.vector.reciprocal(rcp[:], tps[:, 64:65])
                nc.vector.scalar_tensor_tensor(
                    h[:, 512 * b + 64 * jq:512 * b + 64 * jq + 64],
                    tps[:, 0:64], rcp[:],
                    xf[:, 512 * b + 64 * jq:512 * b + 64 * jq + 64],
                    op0=ALU.mult, op1=ALU.add)

    def layer_norm(c):
        """LayerNorm on DVE only; Newton rsqrt batched across the 4 s-tiles."""
        s = st[c]
        h = s["h"]
        mvall = small.tile([128, 8], F32l, tag="mvall", name=f"mv{c}")
        for b in range(4):
            st6 = small.tile([128, 6], F32l, tag="st6", name=f"st6_{c}_{b}")
            nc.vector.bn_stats(st6[:], h[:, 512 * b:512 * (b + 1)])
            nc.vector.bn_aggr(mvall[:, 2 * b:2 * b + 2], st6[:])
        mean4 = mvall[:].rearrange("p (b two) -> p b two", two=2)[:, :, 0]
        var4 = mvall[:].rearrange("p (b two) -> p b two", two=2)[:, :, 1]
        t4 = small.tile([128, 4], F32l, tag="t4", name=f"t4_{c}")
        nc.vector.tensor_scalar_add(t4[:], var4, EPS)
        yi = small.tile([128, 4], I32l, tag="yi", name=f"yi{c}")
        nc.vector.tensor_scalar(yi[:], t4[:].bitcast(I32l), 1, None,
                                op0=ALU.arith_shift_right)
        nc.vector.tensor_scalar(yi[:], yi[:], 0x5F3759DF, -1,
                                op0=ALU.subtract, op1=ALU.mult)
        rstd = small.tile([128, 4], F32l, tag="rstd", name=f"rstd{c}")
        nc.vector.tensor_copy(rstd[:], yi[:].bitcast(F32l))
        y2 = small.tile([128, 4], F32l, tag="y2", name=f"y2_{c}")
        dd = small.tile([128, 4], F32l, tag="dd", name=f"dd{c}")
        for _ in range(3):
            nc.vector.tensor_tensor(y2[:], rstd[:], rstd[:], op=ALU.mult)
            nc.vector.tensor_tensor(y2[:], y2[:], t4[:], op=ALU.mult)
            nc.vector.tensor_scalar(dd[:], y2[:], -0.5, 1.5,
                                    op0=ALU.mult, op1=ALU.add)
            nc.vector.tensor_tensor(rstd[:], rstd[:], dd[:], op=ALU.mult)
        bco = small.tile([128, 4], F32l, tag="bco", name=f"bco{c}")
        nc.vector.tensor_tensor(bco[:], mean4, rstd[:], op=ALU.mult)
        nc.vector.tensor_scalar_mul(bco[:], bco[:], -1.0)
        for b in range(4):
            yt = ypool.tile([128, D], F32l, tag="yt", name=f"yt{c}_{b}")
            nc.vector.tensor_scalar(yt[:], h[:, 512 * b:512 * (b + 1)],
                                    rstd[:, b:b + 1], bco[:, b:b + 1],
                                    op0=ALU.mult, op1=ALU.add)
            nc.vector.tensor_tensor(yt[:], yt[:], gb[:], op=ALU.mult)
            nc.vector.tensor_tensor(yt[:], yt[:], bb[:], op=ALU.add)
            nc.sync.dma_start(y_d[c, 128 * b:128 * (b + 1), :], yt[:])

    # ---- emission schedule: stagger chunk-1 prep into chunk-0's strips so
    # the PE fills ACT-idle gaps with the next chunk's projections.
    prep_load(0)
    consts_early()
    consts_late()
    for t in range(4):
        prep_qkv(0, t)
    strips(0, 0)
    prep_load(1)
    strips(0, 1)
    prep_qk(1, 0, "q")
    prep_qk(1, 0, "k")
    prep_v(1, 0)
    prep_qk(1, 1, "q")
    strips(0, 2)
    prep_qk(1, 1, "k")
    prep_v(1, 1)
    prep_qk(1, 2, "q")
    prep_qk(1, 2, "k")
    strips(0, 3)
    prep_v(1, 2)
    prep_qk(1, 3, "q")
    prep_qk(1, 3, "k")
    prep_v(1, 3)
    layer_norm(0)
    for jp in range(4):
        strips(1, jp)
    layer_norm(1)


def build():
    """Build + compile the Bass module (cached)."""
    if "nc" in _STATE:
        return _STATE["nc"]
    _imports()
    nc = bacc.Bacc("TRN2", target_bir_lowering=False, debug=False,
                   num_devices=N_CORES)
    with tile.TileContext(nc) as tc:
        with ExitStack() as ctx:
            _emit(nc, tc, ctx)
    nc.compile()
    _STATE["nc"] = nc
    return nc


def host_inputs(Wq, bq, Wk, bk, Wv, bv, gamma, beta):
    """Shared per-core constant inputs (everything except x chunks)."""
    bf = ml_dtypes.bfloat16
    base = {
        "wq": np.asarray(Wq, np.float32).astype(bf),
        "wk": np.asarray(Wk, np.float32).astype(bf),
        "wv": np.asarray(Wv, np.float32).astype(bf),
        "bq": np.asarray(bq, np.float32).reshape(1, D).astype(bf),
        "bk": np.asarray(bk, np.float32).reshape(1, D).astype(bf),
        "bv": np.asarray(bv, np.float32).reshape(1, D).astype(bf),
        "ones": np.ones((1, D), bf),
        "idf": np.eye(128, dtype=np.float32),
        "gb": np.broadcast_to(np.asarray(gamma, np.float32), (128, D)).copy(),
        "bb": np.broadcast_to(np.asarray(beta, np.float32), (128, D)).copy(),
    }
    return base


def kernel(x, Wq, bq, Wk, bk, Wv, bv, gamma, beta):
    _imports()
    nc = build()
    x = np.asarray(x, np.float32)
    B, Sfull, Dm = x.shape
    chunks = x.reshape(B * 8, S, D)  # chunk c = (b = c//8, head = c%8)
    bf = ml_dtypes.bfloat16
    base = host_inputs(Wq=Wq, bq=bq, Wk=Wk, bk=bk, Wv=Wv, bv=bv,
                       gamma=gamma, beta=beta)
    in_maps = []
    for i in range(N_CORES):
        xc = np.ascontiguousarray(chunks[2 * i:2 * i + 2])
        m = dict(base)
        m["xc"] = xc
        m["xcb"] = xc.astype(bf)
        in_maps.append(m)
    res = bass_utils.run_bass_kernel_spmd(nc, in_maps, core_ids=list(range(N_CORES)))
    out_chunks = np.empty((B * 8, S, D), np.float32)
    for i in range(N_CORES):
        out_chunks[2 * i:2 * i + 2] = res.results[i]["y"]
    return out_chunks.reshape(B, Sfull, Dm)


# revision 20
# speedup vs baseline: 1.0396x; 1.0072x over previous
"""Trainium2 Bass kernel for fused MHA block (nn_MultiHeadAttention_7636451852747).

Reference math (B=2, S=4096, D=512, H=8, hd=64):
    q = (x @ Wq + bq).reshape(B, H, S, hd)   # torch-style .view, no transpose!
    ... scores = q @ k^T / 8; attn = softmax(scores) @ v -> reshape(B,S,D)
    y = LayerNorm(x + attn) * gamma + beta

Key structural insight: the .view(B,H,S,hd) reshape (without transpose) means
head h of batch b only reads rows [h*512, (h+1)*512) of x[b].  The problem
therefore decomposes into B*H = 16 fully independent [512,512] chunks; each of
the 8 cores processes 2 chunks end-to-end with zero inter-core communication.

Within a chunk (x_c = x[b, h*512:(h+1)*512, :], shape [512, 512]):
    q = x_c Wq + bq viewed as Q[4096, 64] with Q[8s+j, d] = q[s, 64j+d]
    scores^T tiles: S_T[(jk,r)][p, s_q] for nk = 8*(128r+p)+jk, nq = 8*s_q+jq
      = matmul(lhsT=k^T[64jk:+64, 128r:+128], rhs=q^T[64jq:+64, :])
    E = exp(S_T/8) in bf16; attn^T strip = sum over (jk,r) of
      matmul(lhsT=[V_tile | ones], rhs=E) -> [65, 512] psum; row 64 = softmax
      denominator (ones-column trick).  PE-transpose [65,128] blocks back to
      natural layout, divide by denominator, add residual, LayerNorm on DVE
      (Newton rsqrt to keep ACT free for exp, which is the bottleneck engine).
All matmuls are bf16 with fp32 PSUM accumulation.
"""
import os
import numpy as np
import ml_dtypes
from contextlib import ExitStack

BF16 = None  # set in _imports
_STATE = {}


def _imports():
    global bass, bacc, tile, mybir, bass_utils, F32, BF16, I32, ALU, ACTF
    import concourse.bass as bass
    import concourse.bacc as bacc
    import concourse.tile as tile
    from concourse import mybir
    from concourse import bass_utils
    F32 = mybir.dt.float32
    BF16 = mybir.dt.bfloat16
    I32 = mybir.dt.int32
    ALU = mybir.AluOpType
    ACTF = mybir.ActivationFunctionType


N_CORES = 8
CHUNKS_PER_CORE = 2
S = 512          # rows per chunk
D = 512          # model dim
HD = 64          # head dim of the viewed [4096, 64] matrices
NQ = 4096        # sub-rows per chunk (S*D/HD)
EPS = 1e-5


def _emit(nc, tc, ctx):
    F32l, BF16l, I32l = F32, BF16, I32
    x_d = nc.dram_tensor("xc", [CHUNKS_PER_CORE, S, D], F32l, kind="ExternalInput").ap()
    xb_d = nc.dram_tensor("xcb", [CHUNKS_PER_CORE, S, D], BF16l, kind="ExternalInput").ap()
    w_d = {n: nc.dram_tensor(n, [D, D], BF16l, kind="ExternalInput").ap()
           for n in ("wq", "wk", "wv")}
    b_d = {n: nc.dram_tensor(n, [1, D], BF16l, kind="ExternalInput").ap()
           for n in ("bq", "bk", "bv")}
    ones_d = nc.dram_tensor("ones", [1, D], BF16l, kind="ExternalInput").ap()
    idf_d = nc.dram_tensor("idf", [128, 128], F32l, kind="ExternalInput").ap()
    gb_d = nc.dram_tensor("gb", [128, D], F32l, kind="ExternalInput").ap()
    bb_d = nc.dram_tensor("bb", [128, D], F32l, kind="ExternalInput").ap()
    y_d = nc.dram_tensor("y", [CHUNKS_PER_CORE, S, D], F32l, kind="ExternalOutput").ap()

    # pools
    consts = ctx.enter_context(tc.tile_pool(name="consts", bufs=1))
    chunkp = ctx.enter_context(tc.tile_pool(name="chunk", bufs=2))
    epool = ctx.enter_context(tc.tile_pool(name="epool", bufs=8))
    attp = ctx.enter_context(tc.tile_pool(name="attp", bufs=2))
    ypool = ctx.enter_context(tc.tile_pool(name="ypool", bufs=3))
    small = ctx.enter_context(tc.tile_pool(name="small", bufs=4))
    # PSUM budget (8 banks): score 2x[128,1024]=4, attn 2, proj 2 (shared
    # with the finalize transposes via the same tag)
    ps_proj = ctx.enter_context(tc.tile_pool(name="ps_proj", bufs=2, space="PSUM"))
    ps_score = ctx.enter_context(tc.tile_pool(name="ps_score", bufs=2, space="PSUM"))
    ps_attn = ctx.enter_context(tc.tile_pool(name="ps_attn", bufs=2, space="PSUM"))

    # ---- constant tiles (DMAs emitted by _consts_early/_late below so the
    # x-transpose DMAs can go FIRST in the single HWDGE queue: the first
    # projection matmul is gated on x^T, not on the weights)
    w_sb = {n: consts.tile([128, 4 * D], BF16l, tag=n, name=f"w_{n}")
            for n in ("wq", "wk", "wv")}
    b_sb = {n: consts.tile([1, D], BF16l, tag=n, name=f"b_{n}")
            for n in ("bq", "bk", "bv")}
    ones = consts.tile([1, D], BF16l, tag="ones")
    idf = consts.tile([128, 128], F32l, tag="idf")
    gb = consts.tile([128, D], F32l, tag="gb")
    bb = consts.tile([128, D], F32l, tag="bb")

    def consts_early():
        for n in ("wq", "wk"):
            for mt in range(4):
                nc.sync.dma_start(w_sb[n][:, 512 * mt:512 * (mt + 1)],
                                  w_d[n][128 * mt:128 * (mt + 1), :])
        for n in ("bq", "bk"):
            nc.sync.dma_start(b_sb[n][:], b_d[n][:])
        nc.sync.dma_start(ones[:], ones_d[:])

    def consts_late():
        for mt in range(4):
            nc.sync.dma_start(w_sb["wv"][:, 512 * mt:512 * (mt + 1)],
                              w_d["wv"][128 * mt:128 * (mt + 1), :])
        nc.sync.dma_start(b_sb["bv"][:], b_d["bv"][:])
        nc.sync.dma_start(idf[:], idf_d[:])
        nc.sync.dma_start(gb[:], gb_d[:])
        nc.sync.dma_start(bb[:], bb_d[:])

    st = [{} for _ in range(CHUNKS_PER_CORE)]  # per-chunk tile state

    def prep_load(c):
        """DMA x; x^T in one hardware DMA transpose (XBAR, bf16).
        dma_start_transpose into a [p, mt, s] view lands source row m at
        partition m%128 of slab m//128 -- exactly the m-tile-major layout."""
        s = st[c]
        s["xT"] = xT = chunkp.tile([128, 4 * D], BF16l, tag="xT", name=f"xT{c}")
        for mt in range(4):
            nc.sync.dma_start_transpose(
                xT[:, 512 * mt:512 * (mt + 1)], xb_d[c][:, 128 * mt:128 * (mt + 1)])
        s["xf"] = xf = chunkp.tile([128, 4 * D], F32l, tag="xf", name=f"xf{c}")
        for t in range(4):
            nc.sync.dma_start(xf[:, 512 * t:512 * (t + 1)], x_d[c, 128 * t:128 * (t + 1), :])
        s["qT"] = chunkp.tile([128, 4 * D], BF16l, tag="qT", name=f"qT{c}")
        s["qTs"] = chunkp.tile([128, 4 * D], BF16l, tag="qTs", name=f"qTs{c}")
        s["kT"] = chunkp.tile([128, 4 * D], BF16l, tag="kT", name=f"kT{c}")
        s["vp"] = chunkp.tile([128, 4 * 520], BF16l, tag="vp", name=f"vp{c}")
        s["h"] = chunkp.tile([128, 4 * D], F32l, tag="h", name=f"h{c}")

    def prep_qk(c, t, which):
        """One q^T or k^T projection column tile (plus qTs swap for q)."""
        s = st[c]
        xT, qT, qTs, kT = s["xT"], s["qT"], s["qTs"], s["kT"]
        wname, bname, dst = (("wq", "bq", qT) if which == "q" else ("wk", "bk", kT))
        pp = ps_proj.tile([128, D], F32l, tag="proj", name=f"pp{c}_{wname}{t}")
        for mt in range(4):
            nc.tensor.matmul(
                pp[:],
                w_sb[wname][:, 512 * mt + 128 * t:512 * mt + 128 * t + 128],
                xT[:, 512 * mt:512 * (mt + 1)],
                start=(mt == 0), stop=False)
        nc.tensor.matmul(pp[:], b_sb[bname][0:1, 128 * t:128 * (t + 1)],
                         ones[0:1, :], start=False, stop=True)
        nc.vector.tensor_copy(dst[0:64, 512 * t:512 * (t + 1)], pp[0:64, :])
        nc.vector.tensor_copy(dst[64:128, 512 * t:512 * (t + 1)], pp[64:128, :])
        if which == "q":
            nc.sync.dma_start(qTs[64:128, 512 * t:512 * (t + 1)], qT[0:64, 512 * t:512 * (t + 1)])
            nc.sync.dma_start(qTs[0:64, 512 * t:512 * (t + 1)], qT[64:128, 512 * t:512 * (t + 1)])

    def prep_v(c, t):
        s = st[c]
        xT, vp = s["xT"], s["vp"]
        pp = ps_proj.tile([128, D], F32l, tag="proj", name=f"pp{c}_v{t}")
        for mt in range(4):
            nc.tensor.matmul(pp[:], xT[:, 512 * mt + 128 * t:512 * mt + 128 * t + 128],
                             w_sb["wv"][:, 512 * mt:512 * (mt + 1)],
                             start=(mt == 0), stop=False)
        nc.tensor.matmul(pp[:], ones[0:1, 0:128], b_sb["bv"][0:1, :],
                         start=False, stop=True)
        blk = vp[:, 520 * t:520 * (t + 1)].rearrange("p (j c) -> p j c", c=65)
        nc.vector.tensor_copy(blk[:, :, 0:64], pp[:].rearrange("p (j c) -> p j c", c=64))
        nc.vector.memset(blk[:, :, 64], 1.0)

    def prep_qkv(c, t):
        prep_qk(c, t, "q")
        prep_qk(c, t, "k")
        prep_v(c, t)

    def strips(c, jp):
        """One jq-pair: scores (row-packed), 1024-wide exp, attn accumulate,
        transpose back + residual."""
        s = st[c]
        qT, qTs, kT, vp, xf, h = s["qT"], s["qTs"], s["kT"], s["vp"], s["xf"], s["h"]

        def qrhs(jq, par):
            src = qT if (jq % 2) == par else qTs
            return src[64 * par:64 * par + 64, 512 * (jq // 2):512 * (jq // 2) + 512]

        jq0, jq1 = 2 * jp, 2 * jp + 1
        pa = [ps_attn.tile([65, D], F32l, tag="attn", name=f"pa{c}_{jp}_{i}")
              for i in range(2)]
        for r in range(4):
            for jku in range(4):
                jk0, jk1 = 2 * jku, 2 * jku + 1
                koff = 512 * jku + 128 * r
                ps0 = ps_score.tile([128, 2 * D], F32l, tag="sps", name=f"s0_{c}_{jp}_{r}_{jku}")
                ps1 = ps_score.tile([128, 2 * D], F32l, tag="sps", name=f"s1_{c}_{jp}_{r}_{jku}")
                nc.tensor.matmul(ps0[:, 0:512], kT[0:64, koff:koff + 128],
                                 qrhs(jq0, 0), start=True, stop=True,
                                 tile_position=(0, 0))
                nc.tensor.matmul(ps1[:, 0:512], kT[64:128, koff:koff + 128],
                                 qrhs(jq0, 1), start=True, stop=True,
                                 tile_position=(64, 0))
                nc.tensor.matmul(ps0[:, 512:1024], kT[0:64, koff:koff + 128],
                                 qrhs(jq1, 0), start=True, stop=True,
                                 tile_position=(0, 0))
                nc.tensor.matmul(ps1[:, 512:1024], kT[64:128, koff:koff + 128],
                                 qrhs(jq1, 1), start=True, stop=True,
                                 tile_position=(64, 0))
                et0 = epool.tile([128, 2 * D], BF16l, tag="e", name=f"e0_{c}_{jp}_{r}_{jku}")
                et1 = epool.tile([128, 2 * D], BF16l, tag="e", name=f"e1_{c}_{jp}_{r}_{jku}")
                nc.scalar.activation(et0[:], ps0[:], ACTF.Exp, scale=0.125)
                nc.scalar.activation(et1[:], ps1[:], ACTF.Exp, scale=0.125)
                first = (r == 0 and jku == 0)
                last = (r == 3 and jku == 3)
                v0 = vp[:, 520 * r + 65 * jk0:520 * r + 65 * jk0 + 65]
                v1 = vp[:, 520 * r + 65 * jk1:520 * r + 65 * jk1 + 65]
                nc.tensor.matmul(pa[0][:], v0, et0[:, 0:512], start=first,
                                 stop=False, skip_group_check=True)
                nc.tensor.matmul(pa[0][:], v1, et1[:, 0:512], start=False,
                                 stop=last, skip_group_check=True)
                nc.tensor.matmul(pa[1][:], v0, et0[:, 512:1024], start=first,
                                 stop=False, skip_group_check=True)
                nc.tensor.matmul(pa[1][:], v1, et1[:, 512:1024], start=False,
                                 stop=last, skip_group_check=True)
        for half, jq in ((0, jq0), (1, jq1)):
            asb = attp.tile([65, D], F32l, tag="asb", name=f"asb{c}_{jp}_{half}")
            nc.vector.tensor_copy(asb[:], pa[half][:])
            for b in range(4):
                tps = ps_proj.tile([128, 65], F32l, tag="proj", name=f"atr{c}_{jp}_{half}_{b}")
                nc.tensor.transpose(tps[:], asb[0:65, 128 * b:128 * (b + 1)],
                                    idf[0:65, 0:65])
                rcp = small.tile([128, 1], F32l, tag="rcp", name=f"rcp{c}_{jp}_{half}_{b}")
                nc.vector.reciprocal(rcp[:], tps[:, 64:65])
                nc.vector.scalar_tensor_tensor(
                    h[:, 512 * b + 64 * jq:512 * b + 64 * jq + 64],
                    tps[:, 0:64], rcp[:],
                    xf[:, 512 * b + 64 * jq:512 * b + 64 * jq + 64],
                    op0=ALU.mult, op1=ALU.add)

    def layer_norm(c):
        """LayerNorm on DVE only; Newton rsqrt batched across the 4 s-tiles."""
        s = st[c]
        h = s["h"]
        mvall = small.tile([128, 8], F32l, tag="mvall", name=f"mv{c}")
        for b in range(4):
            st6 = small.tile([128, 6], F32l, tag="st6", name=f"st6_{c}_{b}")
            nc.vector.bn_stats(st6[:], h[:, 512 * b:512 * (b + 1)])
            nc.vector.bn_aggr(mvall[:, 2 * b:2 * b + 2], st6[:])
        mean4 = mvall[:].rearrange("p (b two) -> p b two", two=2)[:, :, 0]
        var4 = mvall[:].rearrange("p (b two) -> p b two", two=2)[:, :, 1]
        t4 = small.tile([128, 4], F32l, tag="t4", name=f"t4_{c}")
        nc.vector.tensor_scalar_add(t4[:], var4, EPS)
        yi = small.tile([128, 4], I32l, tag="yi", name=f"yi{c}")
        nc.vector.tensor_scalar(yi[:], t4[:].bitcast(I32l), 1, None,
                                op0=ALU.arith_shift_right)
        nc.vector.tensor_scalar(yi[:], yi[:], 0x5F3759DF, -1,
                                op0=ALU.subtract, op1=ALU.mult)
        rstd = small.tile([128, 4], F32l, tag="rstd", name=f"rstd{c}")
        nc.vector.tensor_copy(rstd[:], yi[:].bitcast(F32l))
        y2 = small.tile([128, 4], F32l, tag="y2", name=f"y2_{c}")
        dd = small.tile([128, 4], F32l, tag="dd", name=f"dd{c}")
        for _ in range(3):
            nc.vector.tensor_tensor(y2[:], rstd[:], rstd[:], op=ALU.mult)
            nc.vector.tensor_tensor(y2[:], y2[:], t4[:], op=ALU.mult)
            nc.vector.tensor_scalar(dd[:], y2[:], -0.5, 1.5,
                                    op0=ALU.mult, op1=ALU.add)
            nc.vector.tensor_tensor(rstd[:], rstd[:], dd[:], op=ALU.mult)
        bco = small.tile([128, 4], F32l, tag="bco", name=f"bco{c}")
        nc.vector.tensor_tensor(bco[:], mean4, rstd[:], op=ALU.mult)
        nc.vector.tensor_scalar_mul(bco[:], bco[:], -1.0)
        for b in range(4):
            yt = ypool.tile([128, D], F32l, tag="yt", name=f"yt{c}_{b}")
            nc.vector.tensor_scalar(yt[:], h[:, 512 * b:512 * (b + 1)],
                                    rstd[:, b:b + 1], bco[:, b:b + 1],
                                    op0=ALU.mult, op1=ALU.add)
            nc.vector.tensor_tensor(yt[:], yt[:], gb[:], op=ALU.mult)
            nc.vector.tensor_tensor(yt[:], yt[:], bb[:], op=ALU.add)
            nc.sync.dma_start(y_d[c, 128 * b:128 * (b + 1), :], yt[:])

    # ---- emission schedule: stagger chunk-1 prep into chunk-0's strips so
    # the PE fills ACT-idle gaps with the next chunk's projections.
    prep_load(0)
    consts_early()
    consts_late()
    for t in range(4):
        prep_qkv(0, t)
    strips(0, 0)
    prep_load(1)
    strips(0, 1)
    prep_qk(1, 0, "q")
    prep_qk(1, 0, "k")
    prep_v(1, 0)
    prep_qk(1, 1, "q")
    strips(0, 2)
    prep_qk(1, 1, "k")
    prep_v(1, 1)
    prep_qk(1, 2, "q")
    prep_qk(1, 2, "k")
    strips(0, 3)
    prep_v(1, 2)
    prep_qk(1, 3, "q")
    prep_qk(1, 3, "k")
    prep_v(1, 3)
    layer_norm(0)
    for jp in range(4):
        strips(1, jp)
    layer_norm(1)


def build():
    """Build + compile the Bass module (cached)."""
    if "nc" in _STATE:
        return _STATE["nc"]
    _imports()
    nc = bacc.Bacc("TRN2", target_bir_lowering=False, debug=False,
                   num_devices=N_CORES)
    with tile.TileContext(nc) as tc:
        with ExitStack() as ctx:
            _emit(nc, tc, ctx)
    nc.compile()
    _STATE["nc"] = nc
    return nc


def host_inputs(Wq, bq, Wk, bk, Wv, bv, gamma, beta):
    """Shared per-core constant inputs (everything except x chunks)."""
    bf = ml_dtypes.bfloat16
    base = {
        "wq": np.asarray(Wq, np.float32).astype(bf),
        "wk": np.asarray(Wk, np.float32).astype(bf),
        "wv": np.asarray(Wv, np.float32).astype(bf),
        "bq": np.asarray(bq, np.float32).reshape(1, D).astype(bf),
        "bk": np.asarray(bk, np.float32).reshape(1, D).astype(bf),
        "bv": np.asarray(bv, np.float32).reshape(1, D).astype(bf),
        "ones": np.ones((1, D), bf),
        "idf": np.eye(128, dtype=np.float32),
        "gb": np.broadcast_to(np.asarray(gamma, np.float32), (128, D)).copy(),
        "bb": np.broadcast_to(np.asarray(beta, np.float32), (128, D)).copy(),
    }
    return base


def kernel(x, Wq, bq, Wk, bk, Wv, bv, gamma, beta):
    _imports()
    nc = build()
    x = np.asarray(x, np.float32)
    B, Sfull, Dm = x.shape
    chunks = x.reshape(B * 8, S, D)  # chunk c = (b = c//8, head = c%8)
    bf = ml_dtypes.bfloat16
    base = host_inputs(Wq=Wq, bq=bq, Wk=Wk, bk=bk, Wv=Wv, bv=bv,
                       gamma=gamma, beta=beta)
    in_maps = []
    for i in range(N_CORES):
        xc = np.ascontiguousarray(chunks[2 * i:2 * i + 2])
        m = dict(base)
        m["xc"] = xc
        m["xcb"] = xc.astype(bf)
        in_maps.append(m)
    res = bass_utils.run_bass_kernel_spmd(nc, in_maps, core_ids=list(range(N_CORES)))
    out_chunks = np.empty((B * 8, S, D), np.float32)
    for i in range(N_CORES):
        out_chunks[2 * i:2 * i + 2] = res.results[i]["y"]
    return out_chunks.reshape(B, Sfull, Dm)
